# revision 59
# baseline (speedup 1.0000x reference)
"""Trainium2 Bass kernel for nn_CompositionBlock (gnn_message_passing).

Data-parallel over batch B=8 across 8 NeuronCores (one sample per core).

Multiply-before-matmul design: the bilinear contractions are done as plain
K-accumulating matmuls over pre-expanded operands
    U[(t',d'), j] = tok[j,t] * dep[j,d]      (stage 1, K = T*D = 8192)
    V[(t',p'), j] = tok[j,t] * h[p,j]        (stage 2, K = T*P = 16384)
with the expansion products computed in SBUF fp16 by VectorE/GpSimd
tensor_tensor at 2x mode (the interleaved (16,8) chunk packing lets one
replicated tok tile serve all chunks of a t-block, and dep/h tiles are
replicated 16x so a whole 2048-col expansion is one DVE op).  This removes
the per-chunk reduce-matmuls and the 1x-mode PSUM-source elementwise of the
earlier design.  h replication is done on-device by 16 selection matmuls +
ScalarE copies.  Final head-scatter is a one-hot matmul.
"""

import json

import numpy as np

B, S, T, D, P = 8, 256, 128, 64, 128
NCORES = 8
JT = S // 128  # token tiles per core


# ----------------------------------------------------------------------------
# Compat: the walrus build in this container accepts at most one sync-wait on
# CTRL-class instructions, but TileContext's tail drain packs several. Split
# any multi-wait instruction into a chain of single-wait clones.
# ----------------------------------------------------------------------------
def _split_multiwait_bir(bir_json_bytes: bytes) -> bytes:
    bir = json.loads(bir_json_bytes)
    for func in bir.get("functions", []):
        for bb in func.get("blocks", []):
            new_instructions = []
            for ins in bb.get("instructions", []):
                si = ins.get("sync_info") or {}
                waits = si.get("on_wait") or []
                if len(waits) > 1:
                    for i, w in enumerate(waits[:-1]):
                        new_instructions.append({
                            "debug": ins.get("debug", 0),
                            "engine": ins["engine"],
                            "ins": [],
                            "name": f"{ins['name']}_w{i}",
                            "opcode": "NoOp",
                            "outs": [],
                            "sync_info": {"on_wait": [w], "on_update": []},
                        })
                    ins["sync_info"] = {
                        "on_wait": [waits[-1]],
                        "on_update": si.get("on_update") or [],
                    }
                new_instructions.append(ins)
            bb["instructions"] = new_instructions
    return json.dumps(bir).encode()


def _install_compat():
    import concourse.bass_utils as bu

    if getattr(bu.compile_bir_kernel, "_multiwait_patched", False):
        return
    orig = bu.compile_bir_kernel

    def patched(bir_json, tmpdir, neff_name="file.neff"):
        return orig(_split_multiwait_bir(bir_json), tmpdir, neff_name)

    patched._multiwait_patched = True
    bu.compile_bir_kernel = patched
    try:
        import concourse.bass2jax as b2j

        if getattr(b2j, "compile_bir_kernel", None) is not None:
            b2j.compile_bir_kernel = patched
    except ImportError:
        pass


_NC_CACHE = {}


def build_nc():
    if "nc" in _NC_CACHE:
        return _NC_CACHE["nc"]
    import concourse.bass as bass
    import concourse.tile as tile
    from concourse import mybir
    from concourse.masks import make_identity

    f32 = mybir.dt.float32
    f16 = mybir.dt.float16
    Alu = mybir.AluOpType
    Act = mybir.ActivationFunctionType

    nc = bass.Bass(trn_type="TRN2")

    # every DMA piece is its own contiguous DRAM tensor (linear reads)
    tok8a_d = nc.dram_tensor("tok8a", [128, 512], f16, kind="ExternalInput")
    tok8b_d = nc.dram_tensor("tok8b", [128, 1536], f16, kind="ExternalInput")
    dep8a_d = nc.dram_tensor("dep8a", [128, 1024], f16, kind="ExternalInput")
    dep8b_d = nc.dram_tensor("dep8b", [128, 1024], f16, kind="ExternalInput")
    w1p_d = [
        nc.dram_tensor(f"w1p{k}", [128, W], f16, kind="ExternalInput")
        for k, W in enumerate([1024, 1024, 2048, 2048, 2048])
    ]
    w2p_d = [
        nc.dram_tensor(f"w2p{k}", [128, W], f16, kind="ExternalInput")
        for k, W in enumerate([2048, 2048, 4096, 4096, 4096])
    ]
    rsel8_d = nc.dram_tensor("rsel8", [128, 512], f16, kind="ExternalInput")
    # iota row + packed per-partition consts (bdep, bcomp, base, heads0,
    # heads1, wr0, wr1) in one contiguous tensor
    iota_d = nc.dram_tensor("iotam", [128, S + 8], f32, kind="ExternalInput")
    c0_d = nc.dram_tensor("c0m", [128, T], f32, kind="ExternalInput")
    out_d = nc.dram_tensor("out", [S, T], f32, kind="ExternalOutput")

    def bcast_row(dram_ap):
        return bass.AP(
            tensor=dram_ap.tensor,
            offset=dram_ap.offset,
            ap=[[0, 128]] + list(dram_ap.ap[1:]),
        )

    def rep_free(ap_slice, n):
        # repeat the free dim of a [128, F] slice n times -> [128, n*F]
        return bass.AP(
            tensor=ap_slice.tensor,
            offset=ap_slice.offset,
            ap=[ap_slice.ap[0]] + [[0, n]] + list(ap_slice.ap[1:]),
        )

    def rep_free2(ap_slice, outer, n, inner):
        # [128, outer*inner] slice -> [128, outer*n*inner]: repeat each
        # inner-block n times (dims: outer, rep, inner)
        return bass.AP(
            tensor=ap_slice.tensor,
            offset=ap_slice.offset,
            ap=[ap_slice.ap[0], [inner, outer], [0, n], [1, inner]],
        )

    with tile.TileContext(nc) as tc:
        with (
            tc.tile_pool(name="consts", bufs=1) as consts,
            tc.tile_pool(name="upool", bufs=5) as upool,
            tc.tile_pool(name="vpool", bufs=5) as vpool,
            tc.tile_pool(name="work", bufs=2) as work,
            tc.tile_pool(name="pstde", bufs=1, space="PSUM") as pstde,
            tc.tile_pool(name="psrepl", bufs=2, space="PSUM") as psrepl,
            tc.tile_pool(name="pscomp", bufs=1, space="PSUM") as pscomp,
            tc.tile_pool(name="pstrans", bufs=2, space="PSUM") as pstrans,
            tc.tile_pool(name="psfin", bufs=2, space="PSUM") as psfin,
        ):
            # ---- input / weight DMAs, ordered for overlap ----
            # sync + scalar are the two parallel HWDGE rings; order by first
            # consumer: consts, then the slices stage-1 needs first, then w1,
            # then w2 streaming under stage-1/2 compute.
            c0_b = consts.tile([128, T], f32)
            iota_b = consts.tile([128, S + 8], f32)
            bdep_c = iota_b[:, S : S + 1]
            bcomp_c = iota_b[:, S + 1 : S + 2]
            base_c = iota_b[:, S + 2 : S + 3]
            headsf_t = [iota_b[:, S + 3 : S + 4], iota_b[:, S + 4 : S + 5]]
            wr_t = [iota_b[:, S + 5 : S + 6], iota_b[:, S + 6 : S + 7]]

            # preload the ScalarE activation table so the first real TANH
            # doesn't eat a ~1.5us ACT_TABLE_LOAD mid-kernel
            warm = consts.tile([128, 1], f32)
            nc.scalar.activation(warm, bdep_c, Act.Tanh)

            tok8_sb = consts.tile([128, 2048], f16)
            dep8_sb = consts.tile([128, 2048], f16)
            w1_sb = consts.tile([128, 8192], f16)
            w2_sb = consts.tile([128, 16384], f16)
            rsel8_sb = consts.tile([128, 512], f16)

            # two HWDGE rings only (queues round-robin, so an extra active
            # ring just steals bandwidth from the critical path); ordered by
            # first consumer
            nc.sync.dma_start(out=w1_sb[:, 0:1024], in_=w1p_d[0][:, :])
            nc.scalar.dma_start(out=tok8_sb[:, 0:512], in_=tok8a_d[:, :])
            nc.sync.dma_start(out=dep8_sb[:, 0:1024], in_=dep8a_d[:, :])
            nc.scalar.dma_start(out=tok8_sb[:, 512:2048], in_=tok8b_d[:, :])
            nc.sync.dma_start(out=dep8_sb[:, 1024:2048], in_=dep8b_d[:, :])
            nc.scalar.dma_start(out=w1_sb[:, 1024:2048], in_=w1p_d[1][:, :])
            nc.sync.dma_start(out=w1_sb[:, 2048:4096], in_=w1p_d[2][:, :])
            nc.scalar.dma_start(out=w1_sb[:, 4096:6144], in_=w1p_d[3][:, :])
            nc.sync.dma_start(out=w1_sb[:, 6144:8192], in_=w1p_d[4][:, :])
            nc.scalar.dma_start(out=iota_b, in_=iota_d[:, :])
            nc.sync.dma_start(out=rsel8_sb, in_=rsel8_d[:, :])
            nc.scalar.dma_start(out=c0_b, in_=c0_d[:, :])
            nc.sync.dma_start(out=w2_sb[:, 0:2048], in_=w2p_d[0][:, :])
            nc.scalar.dma_start(out=w2_sb[:, 2048:4096], in_=w2p_d[1][:, :])
            nc.sync.dma_start(out=w2_sb[:, 4096:8192], in_=w2p_d[2][:, :])
            nc.scalar.dma_start(out=w2_sb[:, 8192:12288], in_=w2p_d[3][:, :])
            nc.sync.dma_start(out=w2_sb[:, 12288:16384], in_=w2p_d[4][:, :])

            # ---- stage 1: tde[p,j] = sum_c W1c.T @ U_c ----
            # first a-block split into two smaller ops so MMs start sooner;
            # the rest are a-pair [128,4096] batches
            tde_ps = pstde.tile([128, S], f32)

            def u_op(c0, nchunks, tok_bcast):
                # in0 = contiguous dep8 stream, in1 = broadcast tok pattern
                u = upool.tile([128, nchunks * 256], f16, name="U", tag="U")
                nc.vector.tensor_tensor(out=u, in0=u_in1(c0, nchunks),
                                        in1=tok_bcast, op=Alu.mult)
                for k in range(nchunks):
                    c = c0 + k
                    nc.tensor.matmul(
                        tde_ps,
                        w1_sb[:, c * 128 : (c + 1) * 128],
                        u[:, k * 256 : (k + 1) * 256],
                        start=(c == 0),
                        stop=(c == 63),
                    )

            def u_in1(c0, nchunks):
                # dep8 column blocks for chunks c0..c0+nchunks (b = c mod 8)
                b0 = c0 % 8
                if nchunks <= 8 - b0:
                    return dep8_sb[:, b0 * 256 : (b0 + nchunks) * 256]
                assert b0 == 0 and nchunks % 8 == 0
                return rep_free(dep8_sb[:, :], nchunks // 8)

            # all (a<4, b<4) chunks first: they only need the first dep8
            # half, which hides the dep8b DMA latency
            for a in range(4):
                u_op(8 * a, 4,
                     rep_free(tok8_sb[:, a * 256 : (a + 1) * 256], 4))
            for a in range(4):
                u_op(8 * a + 4, 4,
                     rep_free(tok8_sb[:, a * 256 : (a + 1) * 256], 4))
            for ap in range(2, 4):
                u_op(
                    16 * ap, 16,
                    rep_free2(tok8_sb[:, ap * 512 : (ap + 1) * 512], 2, 8, 256),
                )

            hT = consts.tile([128, S], f16)
            nc.scalar.activation(hT, tde_ps, Act.Tanh, bias=bdep_c)

            # head-scatter weights: DVE is otherwise idle while h8 replicates
            soh = []
            for jt in range(JT):
                s = consts.tile([128, S], f16, name=f"soh{jt}", tag=f"soh{jt}")
                nc.vector.tensor_scalar(
                    out=s, in0=iota_b[:, 0:S], scalar1=headsf_t[jt],
                    scalar2=wr_t[jt],
                    op0=Alu.is_equal, op1=Alu.mult,
                )
                soh.append(s)

            # ---- replicate h rows into the (t',p') chunk layout ----
            # pairs of selection matmuls share one full PSUM bank; one ScalarE
            # copy evacuates both
            # K=32 selection: rsel32[k, (b2%4)*128+m] = (k == 8*(b2%4)+m%8),
            # rhs = hT rows 32*(b2//4).. -> out[m,j] = hT[8*b2 + m%8, j];
            # replaces the 512KB full selection tensor with 32KB
            h8_sb = consts.tile([128, 4096], f16)
            for pr in range(8):
                rp = psrepl.tile([128, 512], f32, name="rp", tag="rp")
                for k in range(2):
                    b2 = 2 * pr + k
                    g = b2 // 4
                    q = b2 % 4
                    nc.tensor.matmul(
                        rp[:, k * 256 : (k + 1) * 256],
                        rsel8_sb[32 * g : 32 * g + 32, q * 128 : (q + 1) * 128],
                        hT[32 * g : 32 * g + 32, :],
                        tile_position=(32 * g, 0),
                    )
                # all evacuations on ScalarE: with half-major stage-2 order
                # its serial chain stays ahead of DVE's V consumption, and
                # DVE keeps a dense V pipeline
                nc.scalar.copy(h8_sb[:, pr * 512 : (pr + 1) * 512], rp)

            # ---- stage 2: comp[o,j] = sum_c W2c.T @ V_c ----
            # a=0 split in halves so its first V only waits for h8[0:2048];
            # later a's are single [128,4096] ops
            comp_ps = pscomp.tile([128, S], f32)

            def v_op(a, b2lo, nb):
                v = vpool.tile([128, nb * 256], f16, name="V", tag="V")
                nc.vector.tensor_tensor(
                    out=v,
                    in0=h8_sb[:, b2lo * 256 : (b2lo + nb) * 256],
                    in1=rep_free(tok8_sb[:, a * 256 : (a + 1) * 256], nb),
                    op=Alu.mult,
                )
                for k in range(nb):
                    c2 = 16 * a + b2lo + k
                    nc.tensor.matmul(
                        comp_ps,
                        w2_sb[:, c2 * 128 : (c2 + 1) * 128],
                        v[:, k * 256 : (k + 1) * 256],
                        start=(c2 == 0),
                        stop=(c2 == 127),
                    )

            # half-major order: all a-blocks against h8[0:2048] first, so the
            # whole first half of stage 2 only needs two evacuated h8 pairs;
            # the very first ops are even finer so stage 2 starts after a
            # single evacuation
            v_op(0, 0, 2)
            v_op(0, 2, 2)
            v_op(0, 4, 4)
            for a in range(1, 8):
                v_op(a, 0, 8)
            for a in range(8):
                v_op(a, 8, 8)

            specT = work.tile([128, S], f32, name="specT", tag="specT")
            nc.scalar.activation(specT, comp_ps, Act.Tanh, bias=bcomp_c)
            deltaT = consts.tile([128, S], f16)
            nc.vector.tensor_scalar(
                out=deltaT, in0=specT, scalar1=base_c, scalar2=None,
                op0=Alu.subtract,
            )

            ident16 = consts.tile([128, 128], f16)
            make_identity(nc, ident16)

            # ---- transpose delta, head-scatter matmul, bias, store ----
            delta_sb = []
            for jt in range(JT):
                dps = pstrans.tile([128, 128], f16, name="dps", tag="dps")
                nc.tensor.transpose(
                    dps, deltaT[:, jt * 128 : (jt + 1) * 128], ident16
                )
                dsb = consts.tile([128, 128], f16, name=f"delta{jt}", tag=f"delta{jt}")
                if jt % 2 == 0:
                    nc.scalar.copy(dsb, dps)
                else:
                    nc.vector.tensor_copy(dsb, dps)
                delta_sb.append(dsb)

            for ic in range(2):
                fin_ps = psfin.tile([128, T], f32, name="fin", tag="fin")
                for jt in range(JT):
                    nc.tensor.matmul(
                        fin_ps,
                        soh[jt][:, ic * 128 : (ic + 1) * 128],
                        delta_sb[jt],
                        start=(jt == 0),
                        stop=(jt == JT - 1),
                    )
                outsb = work.tile([128, T], f32, name="outsb", tag="outsb")
                nc.vector.tensor_add(outsb, fin_ps, c0_b)
                [nc.sync, nc.scalar][ic].dma_start(
                    out=out_d[ic * 128 : (ic + 1) * 128, :], in_=outsb
                )

    _NC_CACHE["nc"] = nc
    return nc


def prep_core_inputs(token_embeddings, dep_embeddings, dep_heads,
                     W_dep, b_dep, W_comp, b_comp, W_red, b_red):
    f32 = np.float32
    f16 = np.float16
    tok = np.asarray(token_embeddings, dtype=f32)
    dep = np.asarray(dep_embeddings, dtype=f32)
    heads = np.asarray(dep_heads)
    W_dep = np.asarray(W_dep, dtype=f32)
    b_dep = np.asarray(b_dep, dtype=f32)
    W_comp = np.asarray(W_comp, dtype=f32)
    b_comp = np.asarray(b_comp, dtype=f32)
    wr = np.asarray(W_red, dtype=f32)[0]
    b_red = np.asarray(b_red, dtype=f32)

    # w1[r=(t',d'), (c=(a,b), p)] = W_dep[p, 16a+t', 8b+d']
    Z = W_dep.reshape(P, 8, 16, 8, 8)                 # [p, a, t', b, d']
    w1 = np.ascontiguousarray(
        Z.transpose(2, 4, 1, 3, 0).reshape(128, 64 * 128)
    ).astype(f16)
    # w2[r=(t',p'), (c2=(a,b2), o)] = W_comp[o, 16a+t', 8b2+p']
    Z2 = W_comp.reshape(T, 8, 16, 16, 8)              # [o, a, t', b2, p']
    w2 = np.ascontiguousarray(
        Z2.transpose(2, 4, 1, 3, 0).reshape(128, 128 * 128)
    ).astype(f16)
    # rsel8[p, (q, m)] = (p % 32 == 8*q + m % 8), same pattern in each
    # 32-partition group so lhsT/rhs can share a base partition
    kk = np.arange(128)[:, None] % 32
    qq = np.arange(512)[None, :] // 128
    mm = np.arange(512)[None, :] % 128
    rsel8 = (kk == 8 * qq + mm % 8).astype(f16)

    base = np.tanh(b_comp)
    c0 = (base * wr.sum() + b_red[0]).astype(f32)
    iota = np.broadcast_to(np.arange(S, dtype=f32), (128, S))
    headsf = heads.astype(f32).reshape(B, JT, 128)
    wr_t = wr.reshape(JT, 128)

    shared = {
        "rsel8": rsel8,
        "c0m": np.ascontiguousarray(np.broadcast_to(c0, (128, T))),
    }
    w1splits = [0, 1024, 2048, 4096, 6144, 8192]
    for k in range(5):
        shared[f"w1p{k}"] = np.ascontiguousarray(
            w1[:, w1splits[k] : w1splits[k + 1]]
        )
    w2splits = [0, 2048, 4096, 8192, 12288, 16384]
    for k in range(5):
        shared[f"w2p{k}"] = np.ascontiguousarray(
            w2[:, w2splits[k] : w2splits[k + 1]]
        )
    in_maps = []
    for cidx in range(NCORES):
        tokT = np.ascontiguousarray(tok[cidx].T)      # [T, S]
        depT = np.ascontiguousarray(dep[cidx].T)      # [D, S]
        # tok8[r, (a, j)] = tokT[16a + r//8, j]
        X = tokT.reshape(8, 16, S)                    # [a, t', j]
        tok8 = np.repeat(X, 8, axis=1).transpose(1, 0, 2).reshape(128, 8 * S)
        # dep8[r, (b, j)] = depT[8b + r%8, j]
        Y = depT.reshape(8, 8, S)                     # [b, d', j]
        dep8 = np.tile(Y, (1, 16, 1)).transpose(1, 0, 2).reshape(128, 8 * S)
        iotam = np.zeros((128, S + 8), dtype=f32)
        iotam[:, 0:S] = iota
        iotam[:, S + 0] = b_dep
        iotam[:, S + 1] = b_comp
        iotam[:, S + 2] = base
        iotam[:, S + 3] = headsf[cidx][0]
        iotam[:, S + 4] = headsf[cidx][1]
        iotam[:, S + 5] = wr_t[0]
        iotam[:, S + 6] = wr_t[1]
        tok8 = np.ascontiguousarray(tok8).astype(f16)
        dep8 = np.ascontiguousarray(dep8).astype(f16)
        m = dict(shared)
        m["tok8a"] = np.ascontiguousarray(tok8[:, 0:512])
        m["tok8b"] = np.ascontiguousarray(tok8[:, 512:2048])
        m["dep8a"] = np.ascontiguousarray(dep8[:, 0:1024])
        m["dep8b"] = np.ascontiguousarray(dep8[:, 1024:2048])
        m["iotam"] = iotam
        in_maps.append(m)
    return in_maps


def kernel(**inputs) -> np.ndarray:
    _install_compat()
    from concourse.bass_utils import run_bass_kernel_spmd

    nc = build_nc()
    in_maps = prep_core_inputs(**inputs)
    res = run_bass_kernel_spmd(nc, in_maps, core_ids=list(range(NCORES)))
    out = np.stack([res.results[c]["out"] for c in range(NCORES)], axis=0)
    return out.astype(np.float32)


# aliases used by test harness
_build_nc = build_nc
_prep_core_inputs = prep_core_inputs
